# revision 22
# baseline (speedup 1.0000x reference)
"""Trainium2 Bass kernel for the Pointer (sparse_attention) module.

Math (per batch b):
    u  = [state_b | x_l]                      (state broadcast over L)
    s0 = tanh(u @ W0 + b0)                    [L, H]
    s  = s0 @ W1 + b1                         [L]
    s1 = s - INF*(1-mask)                     [L]   (output 2)
    a  = softmax(s1)
    res = sum_l a_l * x_l                     [1, D] (output 1)

Strategy: data-parallel over batch B=32 across 8 cores (4 batches/core, no
collectives).  The state/W0-top/b0 part folds into a per-batch bias
c_b = state_b @ W0[:75] + b0 computed on host (tiny).  X is shipped in bf16 in
two layouts: transposed (for the score MLP, which contracts over D) and
natural+ones-column (for the weighted sum, which contracts over L and also
yields the softmax normalizer Z as column 150).  Scores are computed in
s0^T orientation ([H, L] tiles) so the c_b bias is a per-partition ACT bias
fused into tanh; the W1 dot is a [75,128]x[75,1] matmul per L-tile which puts
scores back into natural [128, Ltile] layout for the mask-add, exp, and
weighted-sum.  Softmax max-subtraction is skipped: |s| <= ~8 guaranteed by
tanh bounds so exp never overflows; masked entries are exactly -1e30 and
exp(-1e30) = 0.  res division by Z happens on host (4x151 floats).
"""

from contextlib import ExitStack

import numpy as np
import ml_dtypes

import concourse.bass as bass
import concourse.tile as tile
from concourse import mybir
from concourse.bass_utils import run_bass_kernel_spmd

BF16 = ml_dtypes.bfloat16
F32 = np.float32

B, L, D_IN, D_STATE, HIDDEN = 32, 4096, 150, 75, 75
INF = 1e30
NCORES = 8
BPC = B // NCORES          # batches per core = 4
NT = L // 128              # L-tiles per batch = 32
NCH = L // 512             # 512-chunks per batch = 8
DH = D_IN // 2             # 75, D split for K<=128
XN_W = D_IN + 1            # natural X width incl ones column = 151

_CACHE: dict = {}


def _build_bass():
    nc = bass.Bass()
    dt = mybir.dt
    # ---- DRAM parameters (per core). 2-D shapes match SBUF layout exactly so
    # every big DMA is per-partition contiguous.
    xt = nc.declare_dram_parameter("xt", [DH, 2 * BPC * L], dt.bfloat16, isOutput=False)
    xn = nc.declare_dram_parameter("xn", [128, BPC * NT * XN_W + 1], dt.bfloat16, isOutput=False)
    wb = nc.declare_dram_parameter("wb", [DH, 256], dt.bfloat16, isOutput=False)
    fb = nc.declare_dram_parameter("fb", [128, BPC + BPC * NT], dt.float32, isOutput=False)
    s1o = nc.declare_dram_parameter("s1o", [128, BPC * NT], dt.float32, isOutput=True)
    acco = nc.declare_dram_parameter("acco", [2, 2 * 2 * XN_W], dt.float32, isOutput=True)

    with tile.TileContext(nc) as tc, ExitStack() as ctx:
        big = ctx.enter_context(tc.tile_pool(name="big", bufs=1))
        s0p = ctx.enter_context(tc.tile_pool(name="s0p", bufs=4))
        pp0 = ctx.enter_context(tc.tile_pool(name="pp0", bufs=3, space="PSUM"))
        pps = ctx.enter_context(tc.tile_pool(name="pps", bufs=1, space="PSUM"))
        ppa = ctx.enter_context(tc.tile_pool(name="ppa", bufs=1, space="PSUM"))

        # static SBUF tensors
        xt_sb = big.tile([DH, 2 * BPC * L], dt.bfloat16)
        xn_sb = big.tile([128, BPC * NT * XN_W + 1], dt.bfloat16)
        wb_sb = big.tile([DH, 256], dt.bfloat16)
        fb_sb = big.tile([128, BPC + BPC * NT], dt.float32)
        w0_sb = wb_sb[:]                     # [75, 256]: two padded halves
        w1_sb = xn_sb[:, BPC * NT * XN_W:]   # [128, 1] bf16, rides in xn blob
        cb_sb = fb_sb[:, 0:BPC]              # [128, BPC]; rows 75+ are zero
        mt_sb = fb_sb[:, BPC:]
        s1_sb = big.tile([128, BPC * NT], dt.float32)
        e_sb = big.tile([128, BPC * NT], dt.bfloat16)
        acc_sb = big.tile([2, 2 * 2 * XN_W], dt.float32)

        warm = big.tile([1, 2], dt.float32)
        nc.vector.memset(warm[:], 0.0)
        nc.scalar.activation(warm[:], warm[:], mybir.ActivationFunctionType.Tanh)
        nc.scalar.activation(warm[:], warm[:], mybir.ActivationFunctionType.Exp)

        # DMA plan: minimize dma_start count (each costs ~0.8us of issue time
        # on its queue) and split issue across the sync HWDGE queue (xt, params)
        # and the gpsimd SWDGE queue (xn, s1 stores) so issue parallelizes.
        # First the head of batch 0 (both halves, first 1024 cols) so mm1 can
        # start ~3us in, then params, then the bulk.
        xt_r = xt[:].rearrange("p (g l) -> p g l", g=2 * BPC)
        xts_r = xt_sb[:].rearrange("p (g l) -> p g l", g=2 * BPC)
        HEAD = 1024
        nc.sync.dma_start(out=xts_r[:, 0:2, 0:HEAD], in_=xt_r[:, 0:2, 0:HEAD])
        nc.sync.dma_start(out=wb_sb[:], in_=wb[:])
        nc.sync.dma_start(out=fb_sb[:], in_=fb[:])
        nc.sync.dma_start(out=xn_sb[:, BPC * NT * XN_W:],
                          in_=xn[:, BPC * NT * XN_W:])
        nc.sync.dma_start(out=xts_r[:, 0:2, HEAD:L], in_=xt_r[:, 0:2, HEAD:L])
        def dma_xt_b(b):
            nc.sync.dma_start(
                out=xt_sb[:, (2 * b) * L:(2 * b + 2) * L],
                in_=xt[:, (2 * b) * L:(2 * b + 2) * L])

        def dma_xn_b(b):
            nc.sync.dma_start(
                out=xn_sb[:, b * NT * XN_W:(b + 1) * NT * XN_W],
                in_=xn[:, b * NT * XN_W:(b + 1) * NT * XN_W])

        # one queue, ordered by need-time: xt(b) before xn(b-1)
        dma_xt_b(1)
        dma_xn_b(0)
        dma_xt_b(2)
        dma_xn_b(1)
        dma_xt_b(3)
        dma_xn_b(2)
        dma_xn_b(3)

        # Software-pipelined emission: PE stream is
        #   mm1(c) ; mm2(c-LAG) ; mm1(c+1) ; mm2(c+1-LAG) ; ... ; wsum pairs
        # so the PE never stalls on the tanh of the chunk it just produced.
        LAG = 2
        NC2 = L // 1024                     # 1024-col double-chunks per batch
        chunks = [(b, j) for b in range(BPC) for j in range(NC2)]
        s0t_tiles: dict = {}
        ps_nat_tiles: dict = {}

        def emit_mm1(b, j):
            off0 = (2 * b) * L + 1024 * j
            off1 = (2 * b + 1) * L + 1024 * j
            # [75, 1024] PSUM tile spans 2 banks; each matmul writes one bank.
            ps0 = pp0.tile([128, 1024], dt.float32, name=f"ps0_{b}_{j}", tag="ps0")
            for h in range(2):
                off = 512 * h
                nc.tensor.matmul(
                    ps0[:, off:off + 512], lhsT=w0_sb[:, 0:128],
                    rhs=xt_sb[:, off0 + off: off0 + off + 512],
                    start=True, stop=False)
                nc.tensor.matmul(
                    ps0[:, off:off + 512], lhsT=w0_sb[:, 128:256],
                    rhs=xt_sb[:, off1 + off: off1 + off + 512],
                    start=False, stop=True)
            s0t = s0p.tile([128, 1024], dt.bfloat16, name=f"s0t_{b}_{j}", tag="s0t")
            nc.scalar.activation(
                s0t[:], ps0[:], mybir.ActivationFunctionType.Tanh,
                bias=cb_sb[:, b:b + 1], scale=1.0)
            s0t_tiles[(b, j)] = s0t

        def emit_mm2(b, j):
            if j == 0:
                ps_nat_tiles[b] = pps.tile([128, NT], dt.float32, name=f"psn_{b}", tag="psn")
            ps_nat = ps_nat_tiles[b]
            s0t = s0t_tiles.pop((b, j))
            for t in range(8):
                nc.tensor.matmul(
                    ps_nat[:, 8 * j + t: 8 * j + t + 1],
                    lhsT=s0t[:, 128 * t:128 * (t + 1)],
                    rhs=w1_sb[:], start=True, stop=True)
            if j == NC2 - 1:
                finish_scores(b)

        def finish_scores(b):
            ps_nat = ps_nat_tiles.pop(b)
            nc.vector.tensor_add(
                s1_sb[:, b * NT:(b + 1) * NT], ps_nat[:],
                mt_sb[:, b * NT:(b + 1) * NT])
            nc.scalar.activation(
                e_sb[:, b * NT:(b + 1) * NT], s1_sb[:, b * NT:(b + 1) * NT],
                mybir.ActivationFunctionType.Exp)
            nc.gpsimd.dma_start(
                out=s1o[:, b * NT:(b + 1) * NT],
                in_=s1_sb[:, b * NT:(b + 1) * NT])

        e_r = e_sb[:].rearrange("p (b t) -> p b t", b=BPC)
        xn_r = xn_sb[:, 0:BPC * NT * XN_W].rearrange("p (b t w) -> p b t w", b=BPC, t=NT)
        wsum_state: dict = {}

        def start_wsum_pair(bp):
            pacc = ppa.tile([2, 2 * XN_W], dt.float32, name=f"pacc_{bp}", tag="pacc")
            wsum_state[bp] = [pacc, 0]

        def emit_wsum_mms(bp, n):
            pacc, t0_ = wsum_state[bp]
            b0 = 2 * bp
            for t in range(t0_, min(t0_ + n, NT)):
                nc.tensor.matmul(
                    pacc[:], lhsT=e_r[:, b0:b0 + 2, t],
                    rhs=xn_r[:, b0:b0 + 2, t, :],
                    start=(t == 0), stop=(t == NT - 1))
            wsum_state[bp][1] = min(t0_ + n, NT)
            if wsum_state[bp][1] == NT:
                nc.vector.tensor_copy(
                    acc_sb[:, bp * 2 * XN_W:(bp + 1) * 2 * XN_W], pacc[:])
                del wsum_state[bp]

        def emit_wsum_pair(bp):
            start_wsum_pair(bp)
            emit_wsum_mms(bp, NT)

        for ci, (b, j) in enumerate(chunks):
            emit_mm1(b, j)
            if ci - LAG >= 0:
                done = chunks[ci - LAG]
                emit_mm2(*done)
                if done == (1, NC2 - 1):
                    emit_wsum_pair(0)   # E(b0, b1) complete
        for cj in range(len(chunks) - LAG, len(chunks)):
            emit_mm2(*chunks[cj])
        emit_wsum_pair(1)
        nc.sync.dma_start(out=acco[:], in_=acc_sb[:])
    return nc


def _legalize_single_wait(nc):
    """Walrus in this toolchain encodes exactly ONE sync wait per 64B
    instruction (NEURON_ISA_TPB_EVENTS has a single wait slot) and refuses
    instructions with more.  Tile's sem assignment can emit several.  Fix up:
    merge same-semaphore waits (max value), then hoist extra waits onto the
    nearest earlier wait-free instruction of the same engine (waits move
    earlier in the engine's stream -> strictly more conservative, still
    correct).  Drains are left alone (walrus lowers them specially)."""
    for blk in nc.m.functions[0].blocks:
        insts = blk.instructions
        patched = []
        changed = False
        for ins in insts:
            tname = type(ins).__name__
            si = ins.sync_info
            waits = list(si.on_wait) if si else []
            if tname in ("InstCall", "InstUnconditionalBranch") \
                    or len(waits) <= 1:
                patched.append(ins)
                continue
            # merge same-sem waits (keep max value)
            merged = {}
            for w in waits:
                k = (w.id, w.ant_name)
                if k not in merged or merged[k].wait_value < w.wait_value:
                    merged[k] = w
            waits = list(merged.values())
            # extras become NoOp instructions just before -> same block point
            for i, w in enumerate(waits[:-1]):
                changed = True
                patched.append(mybir.InstNoOp(
                    name=f"{ins.name}-w{i}", engine=ins.engine, ins=[], outs=[],
                    sync_info=mybir.SyncInfo(on_wait=[w], on_update=[])))
            si.on_wait = waits[-1:]
            patched.append(ins)
        if changed:
            insts.clear()
            insts.extend(patched)


def _get_nc():
    if "nc" not in _CACHE:
        nc = _build_bass()
        _legalize_single_wait(nc)
        _CACHE["nc"] = nc
    return _CACHE["nc"]


def _prep_inputs(inputs, state, c_mask, W0, b0, W1, b1):
    """Host-side shard + layout prep. Returns in_maps for the 8 cores."""
    X = np.ascontiguousarray(inputs, dtype=F32)            # [32, 4096, 150]
    state = np.asarray(state, F32)
    mask = np.asarray(c_mask).astype(bool)
    W0 = np.asarray(W0, F32)
    b0 = np.asarray(b0, F32)
    W1 = np.asarray(W1, F32)
    b1f = F32(np.asarray(b1, F32)[0])

    Xb = X.astype(BF16)                                    # bf16 once
    # xt: [core, 75, (b, half), 4096]
    Xt = Xb.reshape(NCORES, BPC, L, 2, DH)                  # split D -> (half, dh)
    xt_h = np.ascontiguousarray(Xt.transpose(0, 4, 1, 3, 2))  # [c, dh, b, half, L]
    xt_h = xt_h.reshape(NCORES, DH, 2 * BPC * L)
    # xn: [core, 128, (b, t, 151)] with ones column
    Xn = Xb.reshape(NCORES, BPC, NT, 128, D_IN)
    Xn = Xn.transpose(0, 3, 1, 2, 4)                        # [c, p, b, t, d]
    ones = np.ones((NCORES, 128, BPC, NT, 1), BF16)
    xn_h = np.concatenate([Xn, ones], axis=4).reshape(NCORES, 128, BPC * NT * XN_W)
    w1col = np.zeros((NCORES, 128, 1), BF16)
    w1col[:, :HIDDEN, 0] = W1[:, 0].astype(BF16)
    xn_h = np.ascontiguousarray(np.concatenate([xn_h, w1col], axis=2))
    # mask term: b1 + (mask ? 0 : -INF), natural [c, p, (b, t)]
    Mr = mask.reshape(NCORES, BPC, NT, 128).transpose(0, 3, 1, 2)  # [c, p, b, t]
    mt_h = np.where(Mr, b1f, F32(b1f - F32(INF))).astype(F32)
    mt_h = np.ascontiguousarray(mt_h).reshape(NCORES, 128, BPC * NT)
    # weights blob [75, 256]: two zero-padded W0_bot halves [75, 128] each
    wb_h = np.zeros((DH, 256), BF16)
    wb_h[:, 0:HIDDEN] = W0[D_STATE:D_STATE + DH].astype(BF16)
    wb_h[:, 128:128 + HIDDEN] = W0[D_STATE + DH:].astype(BF16)
    wb_h = np.ascontiguousarray(wb_h)
    # f32 blob [core, 128, 4 + 128]: cols 0:4 = cb (rows 0:75), 4: = maskterm
    cvec = state @ W0[:D_STATE] + b0                        # [32, 75] f32
    cb_h = cvec.reshape(NCORES, BPC, HIDDEN).transpose(0, 2, 1)  # [c, 75, 4]
    fb_h = np.zeros((NCORES, 128, BPC + BPC * NT), F32)
    fb_h[:, :HIDDEN, :BPC] = cb_h
    fb_h[:, :, BPC:] = mt_h
    fb_h = np.ascontiguousarray(fb_h)

    in_maps = []
    for c in range(NCORES):
        in_maps.append({
            "xt": xt_h[c], "xn": xn_h[c], "wb": wb_h, "fb": fb_h[c],
        })
    return in_maps


def _postprocess(results):
    s1 = np.empty((B, L), F32)
    res = np.empty((B, 1, D_IN), F32)
    for c in range(NCORES):
        r = results[c]
        s1c = r["s1o"].reshape(128, BPC, NT).transpose(1, 2, 0)   # [b, t, p]
        s1[c * BPC:(c + 1) * BPC] = s1c.reshape(BPC, L)
        acco = r["acco"]                       # [2, 2*2*XN_W]
        for bp in range(2):
            for row in range(2):
                seg = acco[row, bp * 2 * XN_W + row * XN_W:
                           bp * 2 * XN_W + (row + 1) * XN_W]
                res[c * BPC + 2 * bp + row, 0, :] = seg[:D_IN] / seg[D_IN]
    return res, s1


def kernel(inputs, state, c_mask, W0, b0, W1, b1, _trace=False):
    nc = _get_nc()
    in_maps = _prep_inputs(inputs, state, c_mask, W0, b0, W1, b1)
    out = run_bass_kernel_spmd(nc, in_maps, list(range(NCORES)), trace=_trace)
    if _trace:
        _CACHE["last_bkr"] = out
    res, s1 = _postprocess(out.results)
    return res, s1


# revision 24
# speedup vs baseline: 1.0137x; 1.0137x over previous
"""Trainium2 Bass kernel for the Pointer (sparse_attention) module.

Math (per batch b):
    u  = [state_b | x_l]                      (state broadcast over L)
    s0 = tanh(u @ W0 + b0)                    [L, H]
    s  = s0 @ W1 + b1                         [L]
    s1 = s - INF*(1-mask)                     [L]   (output 2)
    a  = softmax(s1)
    res = sum_l a_l * x_l                     [1, D] (output 1)

Strategy: data-parallel over batch B=32 across 8 cores (4 batches/core, no
collectives).  The state/W0-top/b0 part folds into a per-batch bias
c_b = state_b @ W0[:75] + b0 computed on host (tiny).  X is shipped in bf16 in
two layouts: transposed (for the score MLP, which contracts over D) and
natural+ones-column (for the weighted sum, which contracts over L and also
yields the softmax normalizer Z as column 150).  Scores are computed in
s0^T orientation ([H, L] tiles) so the c_b bias is a per-partition ACT bias
fused into tanh; the W1 dot is a [75,128]x[75,1] matmul per L-tile which puts
scores back into natural [128, Ltile] layout for the mask-add, exp, and
weighted-sum.  Softmax max-subtraction is skipped: |s| <= ~8 guaranteed by
tanh bounds so exp never overflows; masked entries are exactly -1e30 and
exp(-1e30) = 0.  res division by Z happens on host (4x151 floats).
"""

from contextlib import ExitStack

import numpy as np
import ml_dtypes

import concourse.bass as bass
import concourse.tile as tile
from concourse import mybir
from concourse.bass_utils import run_bass_kernel_spmd

BF16 = ml_dtypes.bfloat16
F32 = np.float32

B, L, D_IN, D_STATE, HIDDEN = 32, 4096, 150, 75, 75
INF = 1e30
NCORES = 8
BPC = B // NCORES          # batches per core = 4
NT = L // 128              # L-tiles per batch = 32
NCH = L // 512             # 512-chunks per batch = 8
DH = D_IN // 2             # 75, D split for K<=128
XN_W = D_IN + 1            # natural X width incl ones column = 151

_CACHE: dict = {}


def _build_bass():
    nc = bass.Bass()
    dt = mybir.dt
    # ---- DRAM parameters (per core). 2-D shapes match SBUF layout exactly so
    # every big DMA is per-partition contiguous.
    xt = nc.declare_dram_parameter("xt", [DH, 2 * BPC * L], dt.bfloat16, isOutput=False)
    xn = nc.declare_dram_parameter("xn", [128, BPC * NT * XN_W + 1], dt.bfloat16, isOutput=False)
    wb = nc.declare_dram_parameter("wb", [DH, 256], dt.bfloat16, isOutput=False)
    fb = nc.declare_dram_parameter("fb", [128, BPC + BPC * NT], dt.float32, isOutput=False)
    s1o = nc.declare_dram_parameter("s1o", [128, BPC * NT], dt.float32, isOutput=True)
    acco = nc.declare_dram_parameter("acco", [2, 2 * 2 * XN_W], dt.float32, isOutput=True)

    with tile.TileContext(nc) as tc, ExitStack() as ctx:
        big = ctx.enter_context(tc.tile_pool(name="big", bufs=1))
        s0p = ctx.enter_context(tc.tile_pool(name="s0p", bufs=4))
        # PE warm-up spin during the initial DMA wait: big matmuls on a zeroed
        # tile trip the HAM clock gate to 2.4 GHz before real work arrives.
        # Its PSUM pool closes before the main pools open so the bank is
        # reused (the main pools' first use then depends on the spin, which
        # finishes during the DMA wait anyway).
        pwarm = big.tile([128, 512], dt.bfloat16)
        nc.vector.memset(pwarm[:], 0.0)
        with tc.tile_pool(name="pwp", bufs=1, space="PSUM") as pwp:
            pw_ps = pwp.tile([128, 512], dt.float32)
            for _ in range(20):
                nc.tensor.matmul(pw_ps[:], lhsT=pwarm[:, 0:128], rhs=pwarm[:],
                                 start=True, stop=True)
        pp0 = ctx.enter_context(tc.tile_pool(name="pp0", bufs=3, space="PSUM"))
        pps = ctx.enter_context(tc.tile_pool(name="pps", bufs=1, space="PSUM"))
        ppa = ctx.enter_context(tc.tile_pool(name="ppa", bufs=1, space="PSUM"))

        # static SBUF tensors
        xt_sb = big.tile([DH, 2 * BPC * L], dt.bfloat16)
        xn_sb = big.tile([128, BPC * NT * XN_W + 1], dt.bfloat16)
        wb_sb = big.tile([DH, 256], dt.bfloat16)
        fb_sb = big.tile([128, BPC + BPC * NT], dt.float32)
        w0_sb = wb_sb[:]                     # [75, 256]: two padded halves
        w1_sb = xn_sb[:, BPC * NT * XN_W:]   # [128, 1] bf16, rides in xn blob
        cb_sb = fb_sb[:, 0:BPC]              # [128, BPC]; rows 75+ are zero
        mt_sb = fb_sb[:, BPC:]
        s1_sb = big.tile([128, BPC * NT], dt.float32)
        e_sb = big.tile([128, BPC * NT], dt.bfloat16)
        acc_sb = big.tile([2, 2 * 2 * XN_W], dt.float32)

        warm = big.tile([1, 2], dt.float32)
        nc.vector.memset(warm[:], 0.0)
        nc.scalar.activation(warm[:], warm[:], mybir.ActivationFunctionType.Tanh)
        nc.scalar.activation(warm[:], warm[:], mybir.ActivationFunctionType.Exp)
        big = ctx.enter_context(tc.tile_pool(name="big", bufs=1))
        s0p = ctx.enter_context(tc.tile_pool(name="s0p", bufs=4))
        # PE warm-up spin during the initial DMA wait: big matmuls on a zeroed
        # tile trip the HAM clock gate to 2.4 GHz before real work arrives.
        # Its PSUM pool closes before the main pools open so the bank is
        # reused (the main pools' first use then depends on the spin, which
        # finishes during the DMA wait anyway).
        pwarm = big.tile([128, 512], dt.bfloat16)
        nc.vector.memset(pwarm[:], 0.0)
        with tc.tile_pool(name="pwp", bufs=1, space="PSUM") as pwp:
            pw_ps = pwp.tile([128, 512], dt.float32)
            for _ in range(20):
                nc.tensor.matmul(pw_ps[:], lhsT=pwarm[:, 0:128], rhs=pwarm[:],
                                 start=True, stop=True)
        pp0 = ctx.enter_context(tc.tile_pool(name="pp0", bufs=3, space="PSUM"))
        pps = ctx.enter_context(tc.tile_pool(name="pps", bufs=1, space="PSUM"))
        ppa = ctx.enter_context(tc.tile_pool(name="ppa", bufs=1, space="PSUM"))
        # DMA plan: minimize dma_start count (each costs ~0.8us of issue time
        # on its queue) and split issue across the sync HWDGE queue (xt, params)
        # and the gpsimd SWDGE queue (xn, s1 stores) so issue parallelizes.
        # First the head of batch 0 (both halves, first 1024 cols) so mm1 can
        # start ~3us in, then params, then the bulk.
        xt_r = xt[:].rearrange("p (g l) -> p g l", g=2 * BPC)
        xts_r = xt_sb[:].rearrange("p (g l) -> p g l", g=2 * BPC)
        HEAD = 1024
        nc.sync.dma_start(out=xts_r[:, 0:2, 0:HEAD], in_=xt_r[:, 0:2, 0:HEAD])
        nc.sync.dma_start(out=wb_sb[:], in_=wb[:])
        nc.sync.dma_start(out=fb_sb[:], in_=fb[:])
        nc.sync.dma_start(out=xn_sb[:, BPC * NT * XN_W:],
                          in_=xn[:, BPC * NT * XN_W:])
        nc.sync.dma_start(out=xts_r[:, 0:2, HEAD:L], in_=xt_r[:, 0:2, HEAD:L])
        def dma_xt_b(b):
            nc.sync.dma_start(
                out=xt_sb[:, (2 * b) * L:(2 * b + 2) * L],
                in_=xt[:, (2 * b) * L:(2 * b + 2) * L])

        def dma_xn_b(b):
            nc.sync.dma_start(
                out=xn_sb[:, b * NT * XN_W:(b + 1) * NT * XN_W],
                in_=xn[:, b * NT * XN_W:(b + 1) * NT * XN_W])

        # one queue, ordered by need-time: xt(b) before xn(b-1)
        dma_xt_b(1)
        dma_xn_b(0)
        dma_xt_b(2)
        dma_xn_b(1)
        dma_xt_b(3)
        dma_xn_b(2)
        dma_xn_b(3)

        # Software-pipelined emission: PE stream is
        #   mm1(c) ; mm2(c-LAG) ; mm1(c+1) ; mm2(c+1-LAG) ; ... ; wsum pairs
        # so the PE never stalls on the tanh of the chunk it just produced.
        LAG = 2
        NC2 = L // 1024                     # 1024-col double-chunks per batch
        chunks = [(b, j) for b in range(BPC) for j in range(NC2)]
        s0t_tiles: dict = {}
        ps_nat_tiles: dict = {}

        def emit_mm1(b, j):
            off0 = (2 * b) * L + 1024 * j
            off1 = (2 * b + 1) * L + 1024 * j
            # [75, 1024] PSUM tile spans 2 banks; each matmul writes one bank.
            ps0 = pp0.tile([128, 1024], dt.float32, name=f"ps0_{b}_{j}", tag="ps0")
            for h in range(2):
                off = 512 * h
                nc.tensor.matmul(
                    ps0[:, off:off + 512], lhsT=w0_sb[:, 0:128],
                    rhs=xt_sb[:, off0 + off: off0 + off + 512],
                    start=True, stop=False)
                nc.tensor.matmul(
                    ps0[:, off:off + 512], lhsT=w0_sb[:, 128:256],
                    rhs=xt_sb[:, off1 + off: off1 + off + 512],
                    start=False, stop=True)
            s0t = s0p.tile([128, 1024], dt.bfloat16, name=f"s0t_{b}_{j}", tag="s0t")
            nc.scalar.activation(
                s0t[:], ps0[:], mybir.ActivationFunctionType.Tanh,
                bias=cb_sb[:, b:b + 1], scale=1.0)
            s0t_tiles[(b, j)] = s0t

        def emit_mm2(b, j):
            if j == 0:
                ps_nat_tiles[b] = pps.tile([128, NT], dt.float32, name=f"psn_{b}", tag="psn")
            ps_nat = ps_nat_tiles[b]
            s0t = s0t_tiles.pop((b, j))
            for t in range(8):
                nc.tensor.matmul(
                    ps_nat[:, 8 * j + t: 8 * j + t + 1],
                    lhsT=s0t[:, 128 * t:128 * (t + 1)],
                    rhs=w1_sb[:], start=True, stop=True)
            if j == NC2 - 1:
                finish_scores(b)

        def finish_scores(b):
            ps_nat = ps_nat_tiles.pop(b)
            nc.vector.tensor_add(
                s1_sb[:, b * NT:(b + 1) * NT], ps_nat[:],
                mt_sb[:, b * NT:(b + 1) * NT])
            nc.scalar.activation(
                e_sb[:, b * NT:(b + 1) * NT], s1_sb[:, b * NT:(b + 1) * NT],
                mybir.ActivationFunctionType.Exp)
            nc.gpsimd.dma_start(
                out=s1o[:, b * NT:(b + 1) * NT],
                in_=s1_sb[:, b * NT:(b + 1) * NT])

        e_r = e_sb[:].rearrange("p (b t) -> p b t", b=BPC)
        xn_r = xn_sb[:, 0:BPC * NT * XN_W].rearrange("p (b t w) -> p b t w", b=BPC, t=NT)
        wsum_state: dict = {}

        def start_wsum_pair(bp):
            pacc = ppa.tile([2, 2 * XN_W], dt.float32, name=f"pacc_{bp}", tag="pacc")
            wsum_state[bp] = [pacc, 0]

        def emit_wsum_mms(bp, n):
            pacc, t0_ = wsum_state[bp]
            b0 = 2 * bp
            for t in range(t0_, min(t0_ + n, NT)):
                nc.tensor.matmul(
                    pacc[:], lhsT=e_r[:, b0:b0 + 2, t],
                    rhs=xn_r[:, b0:b0 + 2, t, :],
                    start=(t == 0), stop=(t == NT - 1))
            wsum_state[bp][1] = min(t0_ + n, NT)
            if wsum_state[bp][1] == NT:
                nc.vector.tensor_copy(
                    acc_sb[:, bp * 2 * XN_W:(bp + 1) * 2 * XN_W], pacc[:])
                del wsum_state[bp]

        def emit_wsum_pair(bp):
            start_wsum_pair(bp)
            emit_wsum_mms(bp, NT)

        for ci, (b, j) in enumerate(chunks):
            emit_mm1(b, j)
            if ci - LAG >= 0:
                done = chunks[ci - LAG]
                emit_mm2(*done)
                if done == (1, NC2 - 1):
                    emit_wsum_pair(0)   # E(b0, b1) complete
        for cj in range(len(chunks) - LAG, len(chunks)):
            emit_mm2(*chunks[cj])
        emit_wsum_pair(1)
        nc.sync.dma_start(out=acco[:], in_=acc_sb[:])
    return nc


def _legalize_single_wait(nc):
    """Walrus in this toolchain encodes exactly ONE sync wait per 64B
    instruction (NEURON_ISA_TPB_EVENTS has a single wait slot) and refuses
    instructions with more.  Tile's sem assignment can emit several.  Fix up:
    merge same-semaphore waits (max value), then hoist extra waits onto the
    nearest earlier wait-free instruction of the same engine (waits move
    earlier in the engine's stream -> strictly more conservative, still
    correct).  Drains are left alone (walrus lowers them specially)."""
    for blk in nc.m.functions[0].blocks:
        insts = blk.instructions
        patched = []
        changed = False
        for ins in insts:
            tname = type(ins).__name__
            si = ins.sync_info
            waits = list(si.on_wait) if si else []
            if tname in ("InstCall", "InstUnconditionalBranch") \
                    or len(waits) <= 1:
                patched.append(ins)
                continue
            # merge same-sem waits (keep max value)
            merged = {}
            for w in waits:
                k = (w.id, w.ant_name)
                if k not in merged or merged[k].wait_value < w.wait_value:
                    merged[k] = w
            waits = list(merged.values())
            # extras become NoOp instructions just before -> same block point
            for i, w in enumerate(waits[:-1]):
                changed = True
                patched.append(mybir.InstNoOp(
                    name=f"{ins.name}-w{i}", engine=ins.engine, ins=[], outs=[],
                    sync_info=mybir.SyncInfo(on_wait=[w], on_update=[])))
            si.on_wait = waits[-1:]
            patched.append(ins)
        if changed:
            insts.clear()
            insts.extend(patched)


def _get_nc():
    if "nc" not in _CACHE:
        nc = _build_bass()
        _legalize_single_wait(nc)
        _CACHE["nc"] = nc
    return _CACHE["nc"]


def _prep_inputs(inputs, state, c_mask, W0, b0, W1, b1):
    """Host-side shard + layout prep. Returns in_maps for the 8 cores."""
    X = np.ascontiguousarray(inputs, dtype=F32)            # [32, 4096, 150]
    state = np.asarray(state, F32)
    mask = np.asarray(c_mask).astype(bool)
    W0 = np.asarray(W0, F32)
    b0 = np.asarray(b0, F32)
    W1 = np.asarray(W1, F32)
    b1f = F32(np.asarray(b1, F32)[0])

    Xb = X.astype(BF16)                                    # bf16 once
    # xt: [core, 75, (b, half), 4096]
    Xt = Xb.reshape(NCORES, BPC, L, 2, DH)                  # split D -> (half, dh)
    xt_h = np.ascontiguousarray(Xt.transpose(0, 4, 1, 3, 2))  # [c, dh, b, half, L]
    xt_h = xt_h.reshape(NCORES, DH, 2 * BPC * L)
    # xn: [core, 128, (b, t, 151)] with ones column
    Xn = Xb.reshape(NCORES, BPC, NT, 128, D_IN)
    Xn = Xn.transpose(0, 3, 1, 2, 4)                        # [c, p, b, t, d]
    ones = np.ones((NCORES, 128, BPC, NT, 1), BF16)
    xn_h = np.concatenate([Xn, ones], axis=4).reshape(NCORES, 128, BPC * NT * XN_W)
    w1col = np.zeros((NCORES, 128, 1), BF16)
    w1col[:, :HIDDEN, 0] = W1[:, 0].astype(BF16)
    xn_h = np.ascontiguousarray(np.concatenate([xn_h, w1col], axis=2))
    # mask term: b1 + (mask ? 0 : -INF), natural [c, p, (b, t)]
    Mr = mask.reshape(NCORES, BPC, NT, 128).transpose(0, 3, 1, 2)  # [c, p, b, t]
    mt_h = np.where(Mr, b1f, F32(b1f - F32(INF))).astype(F32)
    mt_h = np.ascontiguousarray(mt_h).reshape(NCORES, 128, BPC * NT)
    # weights blob [75, 256]: two zero-padded W0_bot halves [75, 128] each
    wb_h = np.zeros((DH, 256), BF16)
    wb_h[:, 0:HIDDEN] = W0[D_STATE:D_STATE + DH].astype(BF16)
    wb_h[:, 128:128 + HIDDEN] = W0[D_STATE + DH:].astype(BF16)
    wb_h = np.ascontiguousarray(wb_h)
    # f32 blob [core, 128, 4 + 128]: cols 0:4 = cb (rows 0:75), 4: = maskterm
    cvec = state @ W0[:D_STATE] + b0                        # [32, 75] f32
    cb_h = cvec.reshape(NCORES, BPC, HIDDEN).transpose(0, 2, 1)  # [c, 75, 4]
    fb_h = np.zeros((NCORES, 128, BPC + BPC * NT), F32)
    fb_h[:, :HIDDEN, :BPC] = cb_h
    fb_h[:, :, BPC:] = mt_h
    fb_h = np.ascontiguousarray(fb_h)

    in_maps = []
    for c in range(NCORES):
        in_maps.append({
            "xt": xt_h[c], "xn": xn_h[c], "wb": wb_h, "fb": fb_h[c],
        })
    return in_maps


def _postprocess(results):
    s1 = np.empty((B, L), F32)
    res = np.empty((B, 1, D_IN), F32)
    for c in range(NCORES):
        r = results[c]
        s1c = r["s1o"].reshape(128, BPC, NT).transpose(1, 2, 0)   # [b, t, p]
        s1[c * BPC:(c + 1) * BPC] = s1c.reshape(BPC, L)
        acco = r["acco"]                       # [2, 2*2*XN_W]
        for bp in range(2):
            for row in range(2):
                seg = acco[row, bp * 2 * XN_W + row * XN_W:
                           bp * 2 * XN_W + (row + 1) * XN_W]
                res[c * BPC + 2 * bp + row, 0, :] = seg[:D_IN] / seg[D_IN]
    return res, s1


def kernel(inputs, state, c_mask, W0, b0, W1, b1, _trace=False):
    nc = _get_nc()
    in_maps = _prep_inputs(inputs, state, c_mask, W0, b0, W1, b1)
    out = run_bass_kernel_spmd(nc, in_maps, list(range(NCORES)), trace=_trace)
    if _trace:
        _CACHE["last_bkr"] = out
    res, s1 = _postprocess(out.results)
    return res, s1
